# revision 4
# baseline (speedup 1.0000x reference)
"""MoE feed-forward (8 experts, top-2 routing) on 8 Trainium2 NeuronCores.

Strategy (expert-parallel, host-side dispatch):
  - Host computes the router (gate logits, top-k, softmax) in numpy, then
    gathers each expert's tokens into a dense, padded [C, D] block
    (C = max expert load rounded up to the tile quantum).
  - Core e runs a dense FFN for expert e only: Y_e = P_e * (relu(X_e @ W1[e].T) @ W2[e].T)
    over its gathered tokens. All matmuls run in fp32r (full-rate fp32 PE mode).
  - Host scatter-adds the per-expert outputs back into the [T, D] output
    (ascending expert order, matching the reference's summation order) and
    computes the tiny aux loss from the routing counts.
"""

import sys

sys.path.insert(0, "/opt/trn_rl_repo")

import numpy as np

D_MODEL = 1024
D_FF = 4096
N_EXPERTS = 8
N_CORES = 8
EMA_DECAY = 0.99

# mm1 moving-chunk width (fp32r needs >=256 for full rate, <=512 allowed).
CHUNK = 384

_PROGRAM_CACHE: dict = {}


def _build_program(C: int, D: int, F: int, FB: int, repeat: int = 1):
    """Build + compile the per-core Bass program.

    Per-core tensors:
      xt  [D, C]  ExternalInput   gathered tokens, transposed
      w1t [D, F]  ExternalInput   W1[e].T
      w2t [F, D]  ExternalInput   W2[e].T
      p   [C, 1]  ExternalInput   gate prob per gathered token (0 for pad)
      y   [C, D]  ExternalOutput  p * (relu(xt.T @ w1t) @ w2t)
    """
    import concourse.bacc as bacc
    import concourse.mybir as mybir
    from concourse import tile

    f32 = mybir.dt.float32
    f32r = mybir.dt.float32r
    RELU = mybir.ActivationFunctionType.Relu

    assert C % CHUNK == 0 and C % 128 == 0
    assert D % 128 == 0 and F % FB == 0 and FB % 128 == 0

    KT = D // 128        # k-tiles over d_model
    NFB = F // FB        # d_ff blocks (weights streamed once)
    MF = FB // 128       # d_ff tiles per block
    NTT = C // 128       # token tiles
    NCH = C // CHUNK     # mm1 moving chunks
    ND = D // 512        # mm2 moving chunks of 512

    nc = bacc.Bacc()
    xt_d = nc.dram_tensor("xt", [D, C], f32r, kind="ExternalInput")
    w1t_d = nc.dram_tensor("w1t", [D, F], f32r, kind="ExternalInput")
    w2t_d = nc.dram_tensor("w2t", [F, D], f32r, kind="ExternalInput")
    p_d = nc.dram_tensor("p", [C, 1], f32, kind="ExternalInput")
    y_d = nc.dram_tensor("y", [C, D], f32, kind="ExternalOutput")

    import concourse.bass as bass

    with tile.TileContext(nc) as tc:
        with (
            tc.tile_pool(name="xt", bufs=1) as xt_pool,
            tc.tile_pool(name="w1", bufs=16) as w1_pool,
            tc.tile_pool(name="w2", bufs=8) as w2_pool,
            tc.tile_pool(name="ht", bufs=8) as ht_pool,
            tc.tile_pool(name="yb", bufs=1) as y_pool,
            tc.tile_pool(name="pp", bufs=1) as p_pool,
            tc.tile_pool(name="ps1", bufs=4, space=bass.MemorySpace.PSUM) as ps1,
            tc.tile_pool(name="ps2", bufs=2, space=bass.MemorySpace.PSUM) as ps2,
        ):
            for _rep in range(repeat):
                # Resident activations: XT k-tiles, Y accumulators, P scalars.
                xts = []
                for kk in range(KT):
                    t = xt_pool.tile([128, C], f32r, tag=f"xt{kk}", name=f"xtt{kk}")
                    nc.sync.dma_start(t[:], xt_d[kk * 128:(kk + 1) * 128, :])
                    xts.append(t)
                ys, pps = [], []
                for tt in range(NTT):
                    ys.append(y_pool.tile([128, D], f32, tag=f"y{tt}", name=f"yacc{tt}"))
                    pt = p_pool.tile([128, 1], f32, tag=f"p{tt}", name=f"pt{tt}")
                    nc.sync.dma_start(pt[:], p_d[tt * 128:(tt + 1) * 128, :])
                    pps.append(pt)

                for fb in range(NFB):
                    w1tiles = []
                    for kk in range(KT):
                        t = w1_pool.tile([128, FB], f32r, tag="w1", name="w1tile")
                        nc.sync.dma_start(
                            t[:], w1t_d[kk * 128:(kk + 1) * 128, fb * FB:(fb + 1) * FB]
                        )
                        w1tiles.append(t)
                    w2tiles = []
                    for mf in range(MF):
                        r0 = fb * FB + mf * 128
                        t = w2_pool.tile([128, D], f32r, tag="w2", name="w2tile")
                        nc.sync.dma_start(t[:], w2t_d[r0:r0 + 128, :])
                        w2tiles.append(t)

                    # mm1: Ht[mf] [128, C] = (W1T block).T @ XT, + relu
                    httiles = []
                    for mf in range(MF):
                        ht = ht_pool.tile([128, C], f32r, tag="ht", name="httile")
                        for ch in range(NCH):
                            ph = ps1.tile([128, CHUNK], f32, tag="ph", name="phtile")
                            for kk in range(KT):
                                nc.tensor.matmul(
                                    ph[:],
                                    w1tiles[kk][:, mf * 128:(mf + 1) * 128],
                                    xts[kk][:, ch * CHUNK:(ch + 1) * CHUNK],
                                    start=(kk == 0),
                                    stop=(kk == KT - 1),
                                )
                            nc.scalar.activation(
                                ht[:, ch * CHUNK:(ch + 1) * CHUNK], ph[:], RELU
                            )
                        httiles.append(ht)

                    # mm2: Y[tt] += Ht.T @ W2T block
                    for tt in range(NTT):
                        py = ps2.tile([128, D], f32, tag="py", name="pytile")
                        for mf in range(MF):
                            for nh in range(ND):
                                nc.tensor.matmul(
                                    py[:, nh * 512:(nh + 1) * 512],
                                    httiles[mf][:, tt * 128:(tt + 1) * 128],
                                    w2tiles[mf][:, nh * 512:(nh + 1) * 512],
                                    start=(mf == 0),
                                    stop=(mf == MF - 1),
                                )
                        if fb == 0:
                            nc.vector.tensor_copy(ys[tt][:], py[:])
                        else:
                            nc.vector.tensor_add(ys[tt][:], ys[tt][:], py[:])
                        if fb == NFB - 1:
                            nc.vector.tensor_scalar_mul(ys[tt][:], ys[tt][:], pps[tt][:])
                            nc.sync.dma_start(y_d[tt * 128:(tt + 1) * 128, :], ys[tt][:])

    nc.compile()
    return nc


def _get_program(C: int, D: int, F: int, FB: int, repeat: int = 1):
    key = (C, D, F, FB, repeat)
    if key not in _PROGRAM_CACHE:
        _PROGRAM_CACHE[key] = _build_program(C, D, F, FB, repeat)
    return _PROGRAM_CACHE[key]


def _route(flat_x: np.ndarray, Wg: np.ndarray, k: int):
    """Numpy replica of the reference router. Returns (idx [T,k], probs [T,k])."""
    logits = flat_x @ Wg.T  # [T, E]
    # top-k, ties broken toward the lower index (matches jax.lax.top_k)
    idx = np.argsort(-logits, axis=1, kind="stable")[:, :k]
    scores = np.take_along_axis(logits, idx, axis=1).astype(np.float32)
    m = scores.max(axis=1, keepdims=True)
    e = np.exp(scores - m, dtype=np.float32)
    probs = e / e.sum(axis=1, keepdims=True)
    return idx.astype(np.int64), probs.astype(np.float32)


def _run_spmd(nc, in_maps):
    from concourse.bass_utils import run_bass_kernel_spmd

    return run_bass_kernel_spmd(nc, in_maps, core_ids=list(range(N_CORES)))


def kernel(x, Wg, W1, W2, k):
    x = np.asarray(x, dtype=np.float32)
    Wg = np.asarray(Wg, dtype=np.float32)
    W1 = np.asarray(W1, dtype=np.float32)
    W2 = np.asarray(W2, dtype=np.float32)
    k = int(k)

    B, S, D = x.shape
    T = B * S
    E, F = W1.shape[0], W1.shape[1]
    flat_x = np.ascontiguousarray(x.reshape(T, D))

    idx, probs = _route(flat_x, Wg, k)

    # Per-expert token lists (ascending token order).
    tok_ids, tok_p = [], []
    counts = np.zeros(E, dtype=np.int64)
    for e in range(E):
        sel = idx == e  # [T, k]
        rows = np.nonzero(sel.any(axis=1))[0]
        slot = np.argmax(sel[rows], axis=1)
        tok_ids.append(rows)
        tok_p.append(probs[rows, slot].astype(np.float32))
        counts[e] = len(rows)

    Cmax = int(counts.max())
    C = max(CHUNK, ((Cmax + CHUNK - 1) // CHUNK) * CHUNK)

    nc = _get_program(C, D, F, FB=512)

    in_maps = []
    for e in range(E):
        ids = tok_ids[e]
        xe = np.zeros((C, D), dtype=np.float32)
        xe[: len(ids)] = flat_x[ids]
        pe = np.zeros((C, 1), dtype=np.float32)
        pe[: len(ids), 0] = tok_p[e]
        in_maps.append(
            {
                "xt": np.ascontiguousarray(xe.T),
                "w1t": np.ascontiguousarray(W1[e].T),
                "w2t": np.ascontiguousarray(W2[e].T),
                "p": pe,
            }
        )

    res = _run_spmd(nc, in_maps)

    y = np.zeros((T, D), dtype=np.float32)
    for e in range(E):
        ids = tok_ids[e]
        y[ids] += res.results[e]["y"][: len(ids)]

    # Aux load-balance loss from the routing counts (fp32, reference op order).
    usage = (counts.astype(np.float32) / np.float32(T)).astype(np.float32)
    ema = (np.float32(1.0 - EMA_DECAY) * usage).astype(np.float32)
    p_ = ema / (ema.sum(dtype=np.float32) + np.float32(1e-9))
    aux = np.float32((p_ * p_).sum(dtype=np.float32) * np.float32(E))

    return y.reshape(B, S, D), np.asarray(aux, dtype=np.float32)


# revision 5
# speedup vs baseline: 1.0708x; 1.0708x over previous
"""MoE feed-forward (8 experts, top-2 routing) on 8 Trainium2 NeuronCores.

Strategy (expert-parallel, host-side dispatch):
  - Host computes the router (gate logits, top-k, softmax) in numpy, then
    gathers each expert's tokens into a dense, padded [C, D] block
    (C = max expert load rounded up to the tile quantum).
  - Core e runs a dense FFN for expert e only: Y_e = P_e * (relu(X_e @ W1[e].T) @ W2[e].T)
    over its gathered tokens. All matmuls run in fp32r (full-rate fp32 PE mode).
  - Host scatter-adds the per-expert outputs back into the [T, D] output
    (ascending expert order, matching the reference's summation order) and
    computes the tiny aux loss from the routing counts.
"""

import sys

sys.path.insert(0, "/opt/trn_rl_repo")

import numpy as np

D_MODEL = 1024
D_FF = 4096
N_EXPERTS = 8
N_CORES = 8
EMA_DECAY = 0.99

# mm1 moving-chunk width (fp32r needs >=256 for full rate, <=512 allowed).
CHUNK = 384

_PROGRAM_CACHE: dict = {}

# Tunables for the device program (sweepable via TimelineSim).
DEFAULT_CFG = dict(
    w1_bufs=16,
    w2_bufs=8,
    ht_bufs=8,
    ps1_bufs=4,
    ps2_bufs=2,
    fuse_scale=True,
)


def _build_program(C: int, D: int, F: int, FB: int, repeat: int = 1, cfg: dict | None = None):
    """Build + compile the per-core Bass program.

    Per-core tensors:
      xt  [D, C]  ExternalInput   gathered tokens, transposed
      w1t [D, F]  ExternalInput   W1[e].T
      w2t [F, D]  ExternalInput   W2[e].T
      p   [C, 1]  ExternalInput   gate prob per gathered token (0 for pad)
      y   [C, D]  ExternalOutput  p * (relu(xt.T @ w1t) @ w2t)
    """
    import concourse.bacc as bacc
    import concourse.bass as bass
    import concourse.mybir as mybir
    from concourse import tile

    cfg = {**DEFAULT_CFG, **(cfg or {})}

    f32 = mybir.dt.float32
    f32r = mybir.dt.float32r
    RELU = mybir.ActivationFunctionType.Relu
    MUL = mybir.AluOpType.mult
    ADD = mybir.AluOpType.add

    assert C % CHUNK == 0 and C % 128 == 0
    assert D % 512 == 0 and F % FB == 0 and FB % 128 == 0

    KT = D // 128        # k-tiles over d_model
    NFB = F // FB        # d_ff blocks (weights streamed once)
    MF = FB // 128       # d_ff tiles per block
    NTT = C // 128       # token tiles
    NCH = C // CHUNK     # mm1 moving chunks
    ND = D // 512        # mm2 moving chunks of 512

    nc = bacc.Bacc()
    xt_d = nc.dram_tensor("xt", [D, C], f32r, kind="ExternalInput")
    w1t_d = nc.dram_tensor("w1t", [D, F], f32r, kind="ExternalInput")
    w2t_d = nc.dram_tensor("w2t", [F, D], f32r, kind="ExternalInput")
    p_d = nc.dram_tensor("p", [C, 1], f32, kind="ExternalInput")
    y_d = nc.dram_tensor("y", [C, D], f32, kind="ExternalOutput")

    with tile.TileContext(nc) as tc:
        with (
            tc.tile_pool(name="xt", bufs=1) as xt_pool,
            tc.tile_pool(name="w1", bufs=cfg["w1_bufs"]) as w1_pool,
            tc.tile_pool(name="w2", bufs=cfg["w2_bufs"]) as w2_pool,
            tc.tile_pool(name="ht", bufs=cfg["ht_bufs"]) as ht_pool,
            tc.tile_pool(name="yb", bufs=1) as y_pool,
            tc.tile_pool(name="pp", bufs=1) as p_pool,
            tc.tile_pool(name="ps1", bufs=cfg["ps1_bufs"], space=bass.MemorySpace.PSUM) as ps1,
            tc.tile_pool(name="ps2", bufs=cfg["ps2_bufs"], space=bass.MemorySpace.PSUM) as ps2,
        ):
            for _rep in range(repeat):
                # Resident activations: XT k-tiles, Y accumulators, P scalars.
                xts = []
                for kk in range(KT):
                    t = xt_pool.tile([128, C], f32r, tag=f"xt{kk}", name=f"xtt{kk}")
                    nc.sync.dma_start(t[:], xt_d[kk * 128:(kk + 1) * 128, :])
                    xts.append(t)
                ys, pps = [], []
                for tt in range(NTT):
                    ys.append(y_pool.tile([128, D], f32, tag=f"y{tt}", name=f"yacc{tt}"))
                    pt = p_pool.tile([128, 1], f32, tag=f"p{tt}", name=f"pt{tt}")
                    nc.sync.dma_start(pt[:], p_d[tt * 128:(tt + 1) * 128, :])
                    pps.append(pt)

                for fb in range(NFB):
                    w1tiles = []
                    for kk in range(KT):
                        t = w1_pool.tile([128, FB], f32r, tag="w1", name="w1tile")
                        nc.sync.dma_start(
                            t[:], w1t_d[kk * 128:(kk + 1) * 128, fb * FB:(fb + 1) * FB]
                        )
                        w1tiles.append(t)
                    w2tiles = []
                    for mf in range(MF):
                        r0 = fb * FB + mf * 128
                        t = w2_pool.tile([128, D], f32r, tag="w2", name="w2tile")
                        nc.sync.dma_start(t[:], w2t_d[r0:r0 + 128, :])
                        w2tiles.append(t)

                    # mm1: Ht[mf] [128, C] = (W1T block).T @ XT, + relu
                    httiles = []
                    for mf in range(MF):
                        ht = ht_pool.tile([128, C], f32r, tag="ht", name="httile")
                        for ch in range(NCH):
                            ph = ps1.tile([128, CHUNK], f32, tag="ph", name="phtile")
                            for kk in range(KT):
                                nc.tensor.matmul(
                                    ph[:],
                                    w1tiles[kk][:, mf * 128:(mf + 1) * 128],
                                    xts[kk][:, ch * CHUNK:(ch + 1) * CHUNK],
                                    start=(kk == 0),
                                    stop=(kk == KT - 1),
                                )
                            nc.scalar.activation(
                                ht[:, ch * CHUNK:(ch + 1) * CHUNK], ph[:], RELU
                            )
                        httiles.append(ht)

                    # mm2: Y[tt] += P[tt] * (Ht.T @ W2T block)
                    for tt in range(NTT):
                        py = ps2.tile([128, D], f32, tag="py", name="pytile")
                        for mf in range(MF):
                            for nh in range(ND):
                                nc.tensor.matmul(
                                    py[:, nh * 512:(nh + 1) * 512],
                                    httiles[mf][:, tt * 128:(tt + 1) * 128],
                                    w2tiles[mf][:, nh * 512:(nh + 1) * 512],
                                    start=(mf == 0),
                                    stop=(mf == MF - 1),
                                )
                        if cfg["fuse_scale"]:
                            if fb == 0:
                                nc.vector.tensor_scalar_mul(ys[tt][:], py[:], pps[tt][:])
                            else:
                                nc.vector.scalar_tensor_tensor(
                                    ys[tt][:], py[:], pps[tt][:], ys[tt][:], MUL, ADD
                                )
                            if fb == NFB - 1:
                                nc.sync.dma_start(y_d[tt * 128:(tt + 1) * 128, :], ys[tt][:])
                        else:
                            if fb == 0:
                                nc.vector.tensor_copy(ys[tt][:], py[:])
                            else:
                                nc.vector.tensor_add(ys[tt][:], ys[tt][:], py[:])
                            if fb == NFB - 1:
                                nc.vector.tensor_scalar_mul(ys[tt][:], ys[tt][:], pps[tt][:])
                                nc.sync.dma_start(y_d[tt * 128:(tt + 1) * 128, :], ys[tt][:])

    nc.compile()
    return nc


def _get_program(C: int, D: int, F: int, FB: int, repeat: int = 1):
    key = (C, D, F, FB, repeat)
    if key not in _PROGRAM_CACHE:
        _PROGRAM_CACHE[key] = _build_program(C, D, F, FB, repeat)
    return _PROGRAM_CACHE[key]


def _route(flat_x: np.ndarray, Wg: np.ndarray, k: int):
    """Numpy replica of the reference router. Returns (idx [T,k], probs [T,k])."""
    logits = flat_x @ Wg.T  # [T, E]
    # top-k, ties broken toward the lower index (matches jax.lax.top_k)
    idx = np.argsort(-logits, axis=1, kind="stable")[:, :k]
    scores = np.take_along_axis(logits, idx, axis=1).astype(np.float32)
    m = scores.max(axis=1, keepdims=True)
    e = np.exp(scores - m, dtype=np.float32)
    probs = e / e.sum(axis=1, keepdims=True)
    return idx.astype(np.int64), probs.astype(np.float32)


def _run_spmd(nc, in_maps):
    from concourse.bass_utils import run_bass_kernel_spmd

    return run_bass_kernel_spmd(nc, in_maps, core_ids=list(range(N_CORES)))


def kernel(x, Wg, W1, W2, k):
    x = np.asarray(x, dtype=np.float32)
    Wg = np.asarray(Wg, dtype=np.float32)
    W1 = np.asarray(W1, dtype=np.float32)
    W2 = np.asarray(W2, dtype=np.float32)
    k = int(k)

    B, S, D = x.shape
    T = B * S
    E, F = W1.shape[0], W1.shape[1]
    flat_x = np.ascontiguousarray(x.reshape(T, D))

    idx, probs = _route(flat_x, Wg, k)

    # Per-expert token lists (ascending token order).
    tok_ids, tok_p = [], []
    counts = np.zeros(E, dtype=np.int64)
    for e in range(E):
        sel = idx == e  # [T, k]
        rows = np.nonzero(sel.any(axis=1))[0]
        slot = np.argmax(sel[rows], axis=1)
        tok_ids.append(rows)
        tok_p.append(probs[rows, slot].astype(np.float32))
        counts[e] = len(rows)

    Cmax = int(counts.max())
    C = max(CHUNK, ((Cmax + CHUNK - 1) // CHUNK) * CHUNK)

    nc = _get_program(C, D, F, FB=512)

    in_maps = []
    for e in range(E):
        ids = tok_ids[e]
        xe = np.zeros((C, D), dtype=np.float32)
        xe[: len(ids)] = flat_x[ids]
        pe = np.zeros((C, 1), dtype=np.float32)
        pe[: len(ids), 0] = tok_p[e]
        in_maps.append(
            {
                "xt": np.ascontiguousarray(xe.T),
                "w1t": np.ascontiguousarray(W1[e].T),
                "w2t": np.ascontiguousarray(W2[e].T),
                "p": pe,
            }
        )

    res = _run_spmd(nc, in_maps)

    y = np.zeros((T, D), dtype=np.float32)
    for e in range(E):
        ids = tok_ids[e]
        y[ids] += res.results[e]["y"][: len(ids)]

    # Aux load-balance loss from the routing counts (fp32, reference op order).
    usage = (counts.astype(np.float32) / np.float32(T)).astype(np.float32)
    ema = (np.float32(1.0 - EMA_DECAY) * usage).astype(np.float32)
    p_ = ema / (ema.sum(dtype=np.float32) + np.float32(1e-9))
    aux = np.float32((p_ * p_).sum(dtype=np.float32) * np.float32(E))

    return y.reshape(B, S, D), np.asarray(aux, dtype=np.float32)
